# revision 43
# baseline (speedup 1.0000x reference)
"""Trainium2 Bass kernel for 7x7 local (sparse) attention, SPMD over 8 NeuronCores.

Math (per channel c, pixel p):
    q = w_q @ x, k = w_k @ x            (1x1 convs)
    logit[c,p,(i,j)] = q[c,p] * (kpad[c, p+(i,j)] + rel[c,(i,j)])
    out[c,p] = sum_k softmax_k(logit) * vpad[c, p+(i,j)]
where rel[c,(i,j)] = rel_h[c,i] for c<64, rel_w[c-64,j] for c>=64, and
kpad/vpad are zero-padded by 3 (padded taps contribute exp(q*rel) to the
softmax denominator -- matching the reference exactly).

Sharding: 8 cores = 2 batches x 2 CHANNEL halves x 2 spatial 48-line tiles.
High-half cores store all spatial data TRANSPOSED (W-major), so that the rel
bias always attaches to the stored-ROW tap (rel_h[c,i] for low cores, rel_w
[c,j] for high ones) and one NEFF serves all 8 cores.  Within a core the
128 partitions hold (64 channels x 2 row-halves of 24 lines each).

This layout lets the per-tap "krel = k-window + rel" pass (49 taps worth of
adds per pixel) collapse into SEVEN pre-biased k planes built once per core
(plane_m = k + rel[c,m]; window overlap gives the 7x redundancy win), plus a
1-col-shifted B copy of each for the odd column taps (DVE 2x mode needs
4B-aligned windows).  The logit tensor_tensor then reads plane windows
directly:  logit[c,(dR,dC),p] = q[c,p] * plane_dR[c, p + (dR,dC)].

Per-core device pipeline (21 groups = 3 row-epochs x 7 dC):
  DVE: one batched tensor_tensor per group for logits (q broadcast over the
       7 dR via a stride-0 AP reading the 7 planes via a strided window AP)
       and one for e*v (overlapping-row v AP), both fp16/bf16 2x mode.
  ACT: one exp per group (7 dR batched), and the final recip = exp(-ln(den)).
  PE:  q/k 1x1 convs (64-wide weights, written into both partition halves),
       then one identity-matmul per (dR, 256-px chunk) accumulating
       [num | den] jointly into 5 PSUM banks per epoch.
The emission is software-pipelined by one group so ScalarE's exp[g] overlaps
group g+1's logit TT.  Output epoch = 8 lines x 96 cols: [num|den] for 768 px
= 3 PSUM banks, so two epochs' accumulators coexist and the epoch boundary
(normalize of e vs first matmuls of e+1) fully overlaps.

fp16 is used for the whole logit chain (q, planes, logit): bf16's 8-bit
mantissa on |logit|<=60 costs ~4% absmax error; fp16 keeps it ~1%.
e/prod are bf16 (need range: e up to exp(60)).
"""

import sys

import numpy as np
import ml_dtypes

sys.path.insert(0, "/opt/trn_rl_repo")

import concourse.bass as bass  # noqa: E402
import concourse.tile as tile  # noqa: E402
from concourse import mybir  # noqa: E402
from concourse.vector_clock import ScopedClock  # noqa: E402

F32 = mybir.dt.float32
BF16 = mybir.dt.bfloat16
FP16 = mybir.dt.float16

B, CIN, COUT, H, W, K, PAD = 2, 128, 128, 96, 96, 7, 3
NCORES = 8
RT = 48          # stored-R lines per core
RH = 24          # lines per partition half
SLAB = RH + 2 * PAD   # 30 rows incl halo per half
XSLAB = RT + 2 * PAD  # 54 rows of x slab
PW = 102         # padded plane width (3 + 96 + 3)
WPAD = 104       # v buffer width (96 + 6 pad + 2 spare)
EP_ROWS = 8      # lines per PSUM epoch
EP = RH // EP_ROWS    # 3 epochs
FD = EP_ROWS * W      # 768
PLSTRIDE = SLAB * PW  # 3060 elements per plane
# PSUM chunking: each bank holds [num(256 px) | den(same px)] so one matmul
# accumulates both. 768 px = 3 chunks of 256 -> 3 banks, 3 MMs; two epochs'
# accumulators coexist (3+3 of 8 banks) so the epoch boundary overlaps.
CHUNKS = [(0, 256), (256, 256), (512, 256)]

_cache = {}


def _patch_tile_drain():
    """walrus in this container allows only one sync-wait per instruction;
    split excess waits onto NoOps."""
    if getattr(tile, "_drain_patched", False):
        return

    def _drain_and_barrier(self, tick_clock, wait_clock):
        drain_inst = self.nc.sync.drain()
        wait_clock.add_sem_waits(
            drain_inst.ins, ScopedClock({None: tick_clock.global_clock})
        )
        si = drain_inst.ins.sync_info
        if si is not None and si.on_wait and len(si.on_wait) > 1:
            waits = list(si.on_wait)
            drain_inst.ins.sync_info = mybir.SyncInfo(
                on_wait=waits[:1], on_update=list(si.on_update)
            )
            for w in waits[1:]:
                nop_inst = self.nc.sync.nop()
                nop_inst.ins.sync_info = mybir.SyncInfo(on_wait=[w], on_update=[])
        self.nc.all_engine_barrier()
        assert self.sems is not None
        popped = self.nc._tile_sem_poison_stack.pop()
        assert popped is self._sem_poison
        self.nc.clear_and_free_semaphores(list(self.sems.allocated().values()))
        self.nc.all_engine_barrier()

    tile.TileContext._drain_and_barrier = _drain_and_barrier
    tile._drain_patched = True


def _patch_walrus_max_sem():
    """walrus's codegen emits an end-of-program wipe of the FULL semaphore
    file, one EVENT_SEMAPHORE per sem split across the engines (~260 insts,
    ~6us of pure teardown).  Capping --max-sem-num shrinks the wipe; the
    bass-side sems (150+) are already range-cleared by the Tile teardown."""
    from concourse import bass_utils

    if getattr(bass_utils, "_max_sem_patched", False):
        return

    orig = bass_utils.run_command

    def run_command(argv, **kwargs):
        if argv and "walrus_driver" in str(argv[0]):
            argv = list(argv) + ["--max-sem-num=78"]
        return orig(argv, **kwargs)

    bass_utils.run_command = run_command
    bass_utils._max_sem_patched = True


_split_ctr = [0]


def _split_sync_waits(nc, maxw=1):
    for fn in nc.m.functions:
        for bb in fn.blocks:
            if not any(
                inst.sync_info is not None
                and inst.sync_info.on_wait
                and len(inst.sync_info.on_wait) > maxw
                for inst in bb.instructions
            ):
                continue
            new_list = []
            for inst in bb.instructions:
                si = inst.sync_info
                if si is not None and si.on_wait and len(si.on_wait) > maxw:
                    waits = list(si.on_wait)
                    extra, keep = waits[:-maxw], waits[-maxw:]
                    for i in range(0, len(extra), maxw):
                        _split_ctr[0] += 1
                        nop = mybir.InstNoOp(
                            name=f"splitw-{_split_ctr[0]}", ins=[], outs=[]
                        )
                        nop.engine = inst.engine
                        nop.sync_info = mybir.SyncInfo(
                            on_wait=extra[i : i + maxw], on_update=[]
                        )
                        new_list.append(nop)
                    inst.sync_info = mybir.SyncInfo(
                        on_wait=keep, on_update=list(si.on_update)
                    )
                new_list.append(inst)
            try:
                bb.instructions = new_list
            except Exception:
                bb.instructions.clear()
                bb.instructions.extend(new_list)


def _build():
    _patch_tile_drain()
    nc = bass.Bass("TRN2", target_bir_lowering=False, debug=False)

    # xs carries the two 1x1-conv weight panels in its first 128 columns,
    # then the x slab in CONSUMPTION order: the convs for partition-half rh
    # read slab rows 24rh+r, so the blocks are [h0 rows 0-14 | h1 rows
    # 24-38 | h0 rows 15-29 | h1 rows 39-53] (the 6 halo rows 24-29 are
    # stored twice).  DMA chunk A = weights+blocks 1-2 gates the first
    # half of the convs; chunk B = blocks 3-4 the rest.
    XB = 128          # weight-panel columns before the x slab
    NBR = 15          # slab rows per block
    XSW = XB + 4 * NBR * W
    xs_ext = nc.dram_tensor("xs", [128, XSW], FP16, kind="ExternalInput").ap()
    va_ext = nc.dram_tensor("vbufA", [128, SLAB, WPAD], BF16, kind="ExternalInput").ap()
    vb_ext = nc.dram_tensor("vbufB", [128, SLAB, WPAD], BF16, kind="ExternalInput").ap()
    rel_ext = nc.dram_tensor("relcols", [128, 8], F32, kind="ExternalInput").ap()
    id_ext = nc.dram_tensor("ident", [128, 128], BF16, kind="ExternalInput").ap()
    out_ext = nc.dram_tensor("out", [128, RH * W], F32, kind="ExternalOutput").ap()

    from contextlib import ExitStack

    with tile.TileContext(nc) as tc, ExitStack() as ctx:
        consts = ctx.enter_context(tc.tile_pool(name="consts", bufs=1))
        main = ctx.enter_context(tc.tile_pool(name="main", bufs=1))

        ident = consts.tile([128, 128], BF16)
        relc = consts.tile([128, 8], F32)
        vbufA = main.tile([128, SLAB, WPAD], BF16)
        vbufB = main.tile([128, SLAB, WPAD], BF16)
        q_sbs = [main.tile([128, FD], FP16, name=f"q{e}") for e in range(EP)]
        planesA = main.tile([128, K, SLAB, PW], FP16)
        planesB = main.tile([128, K, SLAB, PW], FP16)

        # ---- preamble: q/k 1x1 convs into the (channel, row-half) layout
        # xs lives in the persistent pool: reusing its space for the loop
        # pools would add a WAR edge from the first logit onto EVERY xs
        # consumer (measured +3us on the first TT).
        xs = main.tile([128, XSW], FP16, name="xs")
        wkT = xs[:, 0:64]
        wqT = xs[:, 64:128]
        scrpad = main.tile([128, 640], FP16, name="scrpad")

        def xoff(rh, r):
            """col offset of 'slab row 24rh+r' for partition-half rh."""
            blk = (1 if rh else 0) if r < NBR else (3 if rh else 2)
            return XB + (blk * NBR + (r - NBR if r >= NBR else r)) * W

        nc.gpsimd.memset(scrpad, 0.0)
        # chunk A (weights + conv-first-half rows), then B, then bulk vA on
        # the sync ring -- strictly in need-order so neither the scheduler's
        # DMA model nor HBM bandwidth can reorder the landings; vbufB rides
        # the otherwise-idle gpsimd ring.
        XMID = XB + 2 * NBR * W
        nc.sync.dma_start(out=xs[:, 0:XMID], in_=xs_ext[:, 0:XMID])
        nc.sync.dma_start(out=xs[:, XMID:XSW], in_=xs_ext[:, XMID:XSW])
        nc.sync.dma_start(out=vbufA, in_=va_ext)
        nc.sync.dma_start(out=vbufB, in_=vb_ext)
        # dummy ACT on scratch zeros (no DMA dep): hoists the ~1.3us
        # ACT_TABLE_LOAD off the critical path; then the two tiny const
        # DMAs ride the scalar ring.
        scr = consts.tile([128, 1], F32)
        nc.scalar.activation(
            out=scr, in_=scrpad[:, 0:1],
            func=mybir.ActivationFunctionType.Exp, bias=0.0, scale=1.0,
        )
        nc.scalar.dma_start(out=relc, in_=rel_ext)
        nc.scalar.dma_start(out=ident, in_=id_ext)

        NPRE = 6
        KW = SLAB * W // NPRE  # 480 = 5 rows
        QW = RH * W // NPRE    # 384 = 4 rows
        # q PSUM pool outlives the k pool: waves 1-2 are emitted INSIDE the
        # main loop (after the first logits) so their PSUM->SBUF casts don't
        # wedge ahead of the first logit TT in the DVE queue.  2 q banks +
        # 2x3 nd banks = 8 once the k pool's 6 are freed.
        ps_preQ = ctx.enter_context(tc.tile_pool(name="ps_preQ", bufs=1, space="PSUM"))
        q_ps = ps_preQ.tile([128, 2, 512], F32, tag="qps")

        def q_wave(w):
            """q conv wave for epoch w; the PSUM->SBUF cast rides the
            DVE (ramp slack) so the q chain never serializes on ACT."""
            for j in range(2):
                m = 2 * w + j
                for rh in range(2):
                    off = xoff(rh, 3 + 4 * m)
                    nc.tensor.matmul(
                        q_ps[64 * rh : 64 * rh + 64, j, 0:QW],
                        wqT,
                        xs[:, off : off + QW],
                        start=True, stop=True,
                    )
            nc.vector.tensor_copy(
                q_sbs[w].rearrange("p (c w) -> p c w", c=2),
                q_ps[:, :, 0:QW],
            )

        with tc.tile_pool(name="ps_pre", bufs=1, space="PSUM") as ps_pre:
            # Tile deps are TILE-granular: one k tile would make the first
            # plane0 ACT wait on the LAST k matmul (gated by DMA chunk B),
            # so the two conv halves get separate PSUM tiles.
            k_psA = ps_pre.tile([128, 3, 512], F32, tag="preA")
            k_psB = ps_pre.tile([128, 3, 512], F32, tag="preB")
            # PE warmup on scratch zeros: ~3.4us of sustained matmuls trips
            # the HAM activity monitor to 2.4GHz before the real convs.
            # Real k MMs overwrite via start=True (cols 480:512 of bank 0
            # are never read).
            for _ in range(8):
                nc.tensor.matmul(
                    k_psA[:, 0, 0:512],
                    scrpad[:, 0:128],
                    scrpad[:, 128:640],
                    start=True, stop=True,
                )

            def k_convs(c0, c1):
                for c in range(c0, c1):
                    kp = k_psA if c < 3 else k_psB
                    for rh in range(2):
                        off = xoff(rh, 5 * c)
                        nc.tensor.matmul(
                            kp[64 * rh : 64 * rh + 64, c % 3, 0:KW],
                            wkT,
                            xs[:, off : off + KW],
                            start=True, stop=True,
                        )

            # PE order: A-gated work first (k c0-2, q wave 0), then B-gated.
            # Waves 1-2 are emitted inside the main loop.
            k_convs(0, 3)
            q_wave(0)
            k_convs(3, NPRE)

            # plane 0 (A and B copies) straight from PSUM with the m=0
            # rel bias; chunk c covers slab rows 5c..5c+4.  Split in
            # row-halves: the first logit only reads plane rows 0..13.
            k_srcA = k_psA[:, :, 0:KW].rearrange("p c (r w) -> p c r w", r=5)
            k_srcB = k_psB[:, :, 0:KW].rearrange("p c (r w) -> p c r w", r=5)
            p0A = planesA[:, 0, :, 3 : 3 + W].rearrange(
                "p (c r) w -> p c r w", c=NPRE
            )
            p0B = planesB[:, 0, :, 2 : 2 + W].rearrange(
                "p (c r) w -> p c r w", c=NPRE
            )
            for cs, src in (((0, 3), k_srcA), ((3, NPRE), k_srcB)):
                for dst in (p0A, p0B):
                    nc.scalar.activation(
                        out=dst[:, cs[0] : cs[1]], in_=src,
                        func=mybir.ActivationFunctionType.Identity,
                        bias=relc[:, 0:1], scale=1.0,
                    )

        def strips(planes, ranges):
            """pad strips for plane 0: value = rel bias (k==0 in the pad).
            (ScalarE copy with a stride-0 src was tried: device INTERNAL
            error -- keep these on the DVE.)"""
            zt = relc[:, 0:1]
            for c0, c1 in ranges:
                dst = planes[:, 0, :, c0:c1]
                src = bass.AP(
                    zt.tensor, zt.offset, [zt.ap[0], [0, SLAB], [0, c1 - c0]]
                )
                nc.vector.tensor_copy(dst, src)

        def chain(planes, r0=0, r1=SLAB, m0=1, m1=K):
            """planes 1..6 = plane0 + (rel[m]-rel[0]), full width incl pads.
            Plane m is only ever read at rows [m, m+24), so clip per m."""
            for m in range(m0, m1):
                a, b = max(r0, m), min(r1, m + RH)
                if a >= b:
                    continue
                nc.vector.tensor_scalar(
                    out=planes[:, m, a:b], in0=planes[:, 0, a:b],
                    scalar1=relc[:, m : m + 1], scalar2=None,
                    op0=mybir.AluOpType.add,
                )

        strips(planesA, [(0, 3), (99, 102)])
        # chain for the FIRST logit in two pieces matching its dR-split:
        # planes 1-3 rows [m,11) gate TT(0,0)a; planes 4-6 rows [m,14)
        # gate TT(0,0)b.  (The remainders ride inside the loop.)
        chain(planesA, 0, 11, 1, 4)

        # ---- main loop: 3 epochs x 7 dC groups, software-pipelined by one
        # group so ScalarE's EXP[g] runs while group g+1's logits build.
        lgp = ctx.enter_context(tc.tile_pool(name="lgp", bufs=2))
        ep_pool = ctx.enter_context(tc.tile_pool(name="ep", bufs=3))
        outp = ctx.enter_context(tc.tile_pool(name="outp", bufs=1))
        ps_loop = ctx.enter_context(tc.tile_pool(name="ps_loop", bufs=2, space="PSUM"))

        nd_tiles = {}

        def phase_logit(e, dC, dR_splits=None):
            """logit[dR] = q * plane_dR window (one TT for all 7 dR, or one
            per dR-span for the first group so work starts before the full
            plane chain is built)."""
            P = dC & 1
            dC2 = dC - P
            planes = planesB if P else planesA
            lg = lgp.tile([128, K, FD], FP16, tag="lg")
            pl = planes[:, 0]
            qs = q_sbs[e][:, 0:FD]
            for m0, m1 in dR_splits or [(0, K)]:
                in1 = bass.AP(
                    pl.tensor,
                    pl.offset + m0 * (PLSTRIDE + PW) + EP_ROWS * e * PW + dC2,
                    [pl.ap[0], [PLSTRIDE + PW, m1 - m0], [PW, EP_ROWS], [1, W]],
                )
                q_bc = bass.AP(
                    qs.tensor, qs.offset,
                    [qs.ap[0], [0, m1 - m0], [W, EP_ROWS], [1, W]],
                )
                nc.vector.tensor_tensor(
                    out=lg[:, m0:m1].rearrange("p k (r w) -> p k r w", r=EP_ROWS),
                    in0=q_bc, in1=in1, op=mybir.AluOpType.mult,
                )
                if dR_splits and m1 < K:
                    # build the next span's plane rows before its TT
                    chain(planesA, 0, 14, m1, K)
            return lg

        # NOTE: offloading e*v taps to GpSimd was tried and REGRESSED badly:
        # GpSimd shares SBUF ports with the DVE, so concurrent GpSimd tensor
        # ops knock the DVE's tensor_tensor out of 2x mode (~2x slowdown).
        GP_TAPS = 0  # dR taps of the e*v product offloaded to GpSimd

        def phase_rest(e, dC, lg, split=False):
            """exp, e*v, and the accumulate matmuls for group (e, dC). With
            split=True (the very last group) run in two dR-halves so the
            tail matmuls/normalize start earlier."""
            P = dC & 1
            dC2 = dC - P
            vbuf = vbufB if P else vbufA
            ept = ep_pool.tile([128, K, 2, FD], BF16, tag="ept")
            if dC == 0:
                # one PSUM tile per 256-px chunk: deps are tile-granular,
                # so chunk c's normalize must not wait on other chunks' MMs
                nd_tiles[e] = [
                    ps_loop.tile([128, 512], F32, tag=f"nd{c}", name=f"nd{e}_{c}")
                    for c in range(len(CHUNKS))
                ]
            nd_ps = nd_tiles[e]
            rowstep = vbuf.ap[1][0]

            def prod(i0, i1, eng):
                vbase = vbuf[
                    :, EP_ROWS * e + i0 : EP_ROWS * e + i0 + 1, dC2 : dC2 + W
                ]
                v_ov = bass.AP(
                    vbase.tensor, vbase.offset,
                    [vbase.ap[0], [rowstep, i1 - i0], [rowstep, EP_ROWS], [1, W]],
                )
                eng.tensor_tensor(
                    out=ept[:, i0:i1, 0, :].rearrange(
                        "p k (r w) -> p k r w", r=EP_ROWS
                    ),
                    in0=ept[:, i0:i1, 1, :].rearrange(
                        "p k (r w) -> p k r w", r=EP_ROWS
                    ),
                    in1=v_ov, op=mybir.AluOpType.mult,
                )

            if split:
                # tail groups: per-tap prods with the tap's matmuls emitted
                # right behind, so the PE streams close behind the DVE
                # instead of draining 21 matmuls after the last prod.  The
                # ept tile's last-writer dep then points at THIS tap's prod.
                for h0, h1 in ((0, 4), (4, K)):
                    nc.scalar.activation(
                        out=ept[:, h0:h1, 1, :], in_=lg[:, h0:h1, :],
                        func=mybir.ActivationFunctionType.Exp,
                        bias=0.0, scale=1.0,
                    )
                    for i in range(h0, h1):
                        prod(i, i + 1, nc.vector)
                        for c, (px0, cw) in enumerate(CHUNKS):
                            nc.tensor.matmul(
                                nd_ps[c][:, 0 : 2 * cw], ident,
                                ept[:, i, :, px0 : px0 + cw],
                                start=(dC == 0 and i == 0),
                                stop=(dC == K - 1 and i == K - 1),
                            )
                return
            nc.scalar.activation(
                out=ept[:, :, 1, :], in_=lg,
                func=mybir.ActivationFunctionType.Exp, bias=0.0, scale=1.0,
            )
            prod(0, K, nc.vector)
            for i in range(K):
                for c, (px0, cw) in enumerate(CHUNKS):
                    nc.tensor.matmul(
                        nd_ps[c][:, 0 : 2 * cw], ident,
                        ept[:, i, :, px0 : px0 + cw],
                        start=(dC == 0 and i == 0),
                        stop=(dC == K - 1 and i == K - 1),
                    )

        def normalize(e, per_chunk=False):
            """out = num * exp(-ln(den)); [num|den] interleaved 256s.
            Always per 256-px chunk: each chunk's ln only waits ITS OWN
            accumulator tile's stop-matmul, so normalize+DMA overlap the
            remaining chunks' matmuls."""
            nd_ps = nd_tiles[e]
            NC = len(CHUNKS)
            lnden = outp.tile([128, FD], F32, tag="lnden")
            recip = outp.tile([128, FD], F32, tag="recip")
            out_sb = outp.tile([128, FD], F32, tag="out_sb")
            for c in range(NC):
                nc.scalar.activation(
                    out=lnden[:, c * 256 : (c + 1) * 256],
                    in_=nd_ps[c][:, 256:512],
                    func=mybir.ActivationFunctionType.Ln, bias=0.0, scale=1.0,
                )
                nc.scalar.activation(
                    out=recip[:, c * 256 : (c + 1) * 256],
                    in_=lnden[:, c * 256 : (c + 1) * 256],
                    func=mybir.ActivationFunctionType.Exp, bias=0.0, scale=-1.0,
                )
                nc.vector.tensor_tensor(
                    out=out_sb[:, c * 256 : (c + 1) * 256],
                    in0=nd_ps[c][:, 0:256],
                    in1=recip[:, c * 256 : (c + 1) * 256],
                    op=mybir.AluOpType.mult,
                )
                nc.sync.dma_start(
                    out=out_ext[:, e * FD + c * 256 : e * FD + (c + 1) * 256],
                    in_=out_sb[:, c * 256 : (c + 1) * 256],
                )

        groups = [(e, dC) for e in range(EP) for dC in range(K)]
        pending = None  # (e, dC, lg) whose exp/prod/MMs are not yet emitted
        for e, dC in groups:
            lg = phase_logit(
                e, dC, dR_splits=[(0, 4), (4, K)] if (e, dC) == (0, 0) else None
            )
            if (e, dC) == (0, 0):
                # B planes are first needed by group (0,1); building them
                # here overlaps the chain with group (0,0)'s exp.
                strips(planesB, [(0, 2), (98, 102)])
                chain(planesB, 0, 14)
                q_wave(1)
            elif (e, dC) == (0, 1):
                # A-plane remainders (needed from epoch 1 on)
                chain(planesA, 11, SLAB, 1, 4)
                chain(planesA, 14, SLAB, 4, K)
                q_wave(2)
            elif (e, dC) == (0, 2):
                chain(planesB, 14, SLAB)
            if pending is not None:
                pe, pc, plg = pending
                phase_rest(pe, pc, plg, split=(pe == EP - 1 and pc >= K - 2))
                # normalize(e-1) is deferred until after group (e,0)'s rest:
                # its ln/exp would otherwise sit in the ACT queue AHEAD of
                # exp(e,0) and stall the next epoch's first e*v (~0.8us).
                if pc == 0 and pe > 0:
                    normalize(pe - 1)
            pending = (e, dC, lg)
        pe, pc, plg = pending
        phase_rest(pe, pc, plg, split=True)
        normalize(pe, per_chunk=True)

    _split_sync_waits(nc)
    return nc


def _host_prep(x, v, w_q, w_k, rel_h, rel_w):
    """Build the 8 per-core input maps (numpy only)."""
    x = np.asarray(x, np.float32)
    v = np.asarray(v, np.float32)
    w_q = np.asarray(w_q, np.float32)
    w_k = np.asarray(w_k, np.float32)
    rel_h = np.asarray(rel_h, np.float32).reshape(64, K)   # [c, i]
    rel_w = np.asarray(rel_w, np.float32).reshape(64, K)   # [c, j]

    ident = np.eye(128, dtype=np.float32).astype(ml_dtypes.bfloat16)

    in_maps = []
    for ci in range(NCORES):
        b, rest = divmod(ci, 4)
        half, t = divmod(rest, 2)
        ch0 = 64 * half
        if half == 0:
            xf = x[b]                          # [128, R=h, C=w]
            vf = v[b, ch0 : ch0 + 64]
            relv = rel_h                       # [c, m] with m = dR
        else:
            xf = np.ascontiguousarray(x[b].transpose(0, 2, 1))   # R=w, C=h
            vf = np.ascontiguousarray(v[b, ch0 : ch0 + 64].transpose(0, 2, 1))
            relv = rel_w

        R0 = RT * t
        # x slab: stored rows R0-3 .. R0+50, zero beyond the image
        xs = np.zeros((128, XSLAB, W), np.float32)
        glo, ghi = max(0, R0 - PAD), min(96, R0 + RT + PAD)
        xs[:, glo - (R0 - PAD) : ghi - (R0 - PAD), :] = xf[:, glo:ghi, :]

        # v family buffer: partition p = c + 64*rh
        vs = np.zeros((2, 64, SLAB, WPAD), np.float32)
        for rh in range(2):
            r0 = R0 + RH * rh
            lo, hi = max(0, r0 - PAD), min(96, r0 + RH + PAD)
            vs[rh, :, lo - (r0 - PAD) : hi - (r0 - PAD), PAD : PAD + W] = (
                vf[:, lo:hi, :]
            )
        vbufA = vs.reshape(128, SLAB, WPAD)
        vbufB = np.zeros_like(vbufA)
        vbufB[:, :, : WPAD - 1] = vbufA[:, :, 1:]

        relc = np.zeros((128, 8), np.float32)
        rv = np.concatenate([relv, relv], axis=0)          # [128, 7], p=c+64rh
        relc[:, 0] = rv[:, 0]
        relc[:, 1:K] = rv[:, 1:K] - rv[:, 0:1]

        # blocks: [weights | h0 slab rows 0-14 | h1 rows 24-38 |
        #          h0 rows 15-29 | h1 rows 39-53]
        xs_pack = np.empty((128, 128 + 4 * 15 * W), np.float16)
        xs_pack[:, 0:64] = w_k[ch0 : ch0 + 64].T.astype(np.float16)
        xs_pack[:, 64:128] = w_q[ch0 : ch0 + 64].T.astype(np.float16)
        xs16 = xs.astype(np.float16)
        for b, r0 in enumerate((0, 24, 15, 39)):
            xs_pack[:, 128 + b * 15 * W : 128 + (b + 1) * 15 * W] = xs16[
                :, r0 : r0 + 15, :
            ].reshape(128, 15 * W)
        in_maps.append(
            {
                "xs": np.ascontiguousarray(xs_pack),
                "vbufA": np.ascontiguousarray(vbufA.astype(ml_dtypes.bfloat16)),
                "vbufB": np.ascontiguousarray(vbufB.astype(ml_dtypes.bfloat16)),
                "relcols": relc,
                "ident": ident,
            }
        )
    return in_maps


def kernel(x, v, w_q, w_k, rel_h, rel_w, trace=False, tmpdir=None):
    from concourse.bass_utils import run_bass_kernel_spmd

    if "nc" not in _cache:
        _cache["nc"] = _build()
    nc = _cache["nc"]
    in_maps = _host_prep(x, v, w_q, w_k, rel_h, rel_w)
    res = run_bass_kernel_spmd(
        nc, in_maps, list(range(NCORES)), trace=trace, tmpdir=tmpdir
    )
    out = np.zeros((B, COUT, H, W), np.float32)
    for ci in range(NCORES):
        b, rest = divmod(ci, 4)
        half, t = divmod(rest, 2)
        ch0, R0 = 64 * half, RT * t
        a = res.results[ci]["out"].reshape(2, 64, EP, EP_ROWS, W)
        lines = a.transpose(1, 0, 2, 3, 4).reshape(64, RT, W)  # [c, line, C]
        if half == 0:
            out[b, ch0 : ch0 + 64, R0 : R0 + RT, :] = lines
        else:
            out[b, ch0 : ch0 + 64, :, R0 : R0 + RT] = lines.transpose(0, 2, 1)
    kernel.last_exec_time_ns = res.exec_time_ns
    kernel.last_results = res
    return out



# revision 44
# speedup vs baseline: 1.0036x; 1.0036x over previous
"""Trainium2 Bass kernel for 7x7 local (sparse) attention, SPMD over 8 NeuronCores.

Math (per channel c, pixel p):
    q = w_q @ x, k = w_k @ x            (1x1 convs)
    logit[c,p,(i,j)] = q[c,p] * (kpad[c, p+(i,j)] + rel[c,(i,j)])
    out[c,p] = sum_k softmax_k(logit) * vpad[c, p+(i,j)]
where rel[c,(i,j)] = rel_h[c,i] for c<64, rel_w[c-64,j] for c>=64, and
kpad/vpad are zero-padded by 3 (padded taps contribute exp(q*rel) to the
softmax denominator -- matching the reference exactly).

Sharding: 8 cores = 2 batches x 2 CHANNEL halves x 2 spatial 48-line tiles.
High-half cores store all spatial data TRANSPOSED (W-major), so that the rel
bias always attaches to the stored-ROW tap (rel_h[c,i] for low cores, rel_w
[c,j] for high ones) and one NEFF serves all 8 cores.  Within a core the
128 partitions hold (64 channels x 2 row-halves of 24 lines each).

This layout lets the per-tap "krel = k-window + rel" pass (49 taps worth of
adds per pixel) collapse into SEVEN pre-biased k planes built once per core
(plane_m = k + rel[c,m]; window overlap gives the 7x redundancy win), plus a
1-col-shifted B copy of each for the odd column taps (DVE 2x mode needs
4B-aligned windows).  The logit tensor_tensor then reads plane windows
directly:  logit[c,(dR,dC),p] = q[c,p] * plane_dR[c, p + (dR,dC)].

Per-core device pipeline (21 groups = 3 row-epochs x 7 dC):
  DVE: one batched tensor_tensor per group for logits (q broadcast over the
       7 dR via a stride-0 AP reading the 7 planes via a strided window AP)
       and one for e*v (overlapping-row v AP), both fp16/bf16 2x mode.
  ACT: one exp per group (7 dR batched), and the final recip = exp(-ln(den)).
  PE:  q/k 1x1 convs (64-wide weights, written into both partition halves),
       then one identity-matmul per (dR, 256-px chunk) accumulating
       [num | den] jointly into 5 PSUM banks per epoch.
The emission is software-pipelined by one group so ScalarE's exp[g] overlaps
group g+1's logit TT.  Output epoch = 8 lines x 96 cols: [num|den] for 768 px
= 3 PSUM banks, so two epochs' accumulators coexist and the epoch boundary
(normalize of e vs first matmuls of e+1) fully overlaps.

fp16 is used for the whole logit chain (q, planes, logit): bf16's 8-bit
mantissa on |logit|<=60 costs ~4% absmax error; fp16 keeps it ~1%.
e/prod are bf16 (need range: e up to exp(60)).
"""

import sys

import numpy as np
import ml_dtypes

sys.path.insert(0, "/opt/trn_rl_repo")

import concourse.bass as bass  # noqa: E402
import concourse.tile as tile  # noqa: E402
from concourse import mybir  # noqa: E402
from concourse.vector_clock import ScopedClock  # noqa: E402

F32 = mybir.dt.float32
BF16 = mybir.dt.bfloat16
FP16 = mybir.dt.float16

B, CIN, COUT, H, W, K, PAD = 2, 128, 128, 96, 96, 7, 3
NCORES = 8
RT = 48          # stored-R lines per core
RH = 24          # lines per partition half
SLAB = RH + 2 * PAD   # 30 rows incl halo per half
XSLAB = RT + 2 * PAD  # 54 rows of x slab
PW = 102         # padded plane width (3 + 96 + 3)
WPAD = 104       # v buffer width (96 + 6 pad + 2 spare)
EP_ROWS = 8      # lines per PSUM epoch
EP = RH // EP_ROWS    # 3 epochs
FD = EP_ROWS * W      # 768
PLSTRIDE = SLAB * PW  # 3060 elements per plane
# PSUM chunking: each bank holds [num(256 px) | den(same px)] so one matmul
# accumulates both. 768 px = 3 chunks of 256 -> 3 banks, 3 MMs; two epochs'
# accumulators coexist (3+3 of 8 banks) so the epoch boundary overlaps.
CHUNKS = [(0, 256), (256, 256), (512, 256)]

_cache = {}


def _patch_tile_drain():
    """walrus in this container allows only one sync-wait per instruction;
    split excess waits onto NoOps."""
    if getattr(tile, "_drain_patched", False):
        return

    def _drain_and_barrier(self, tick_clock, wait_clock):
        drain_inst = self.nc.sync.drain()
        wait_clock.add_sem_waits(
            drain_inst.ins, ScopedClock({None: tick_clock.global_clock})
        )
        si = drain_inst.ins.sync_info
        if si is not None and si.on_wait and len(si.on_wait) > 1:
            waits = list(si.on_wait)
            drain_inst.ins.sync_info = mybir.SyncInfo(
                on_wait=waits[:1], on_update=list(si.on_update)
            )
            for w in waits[1:]:
                nop_inst = self.nc.sync.nop()
                nop_inst.ins.sync_info = mybir.SyncInfo(on_wait=[w], on_update=[])
        self.nc.all_engine_barrier()
        assert self.sems is not None
        popped = self.nc._tile_sem_poison_stack.pop()
        assert popped is self._sem_poison
        self.nc.clear_and_free_semaphores(list(self.sems.allocated().values()))
        self.nc.all_engine_barrier()

    tile.TileContext._drain_and_barrier = _drain_and_barrier
    tile._drain_patched = True


def _patch_walrus_max_sem():
    """walrus's codegen emits an end-of-program wipe of the FULL semaphore
    file, one EVENT_SEMAPHORE per sem split across the engines (~260 insts,
    ~6us of pure teardown).  Capping --max-sem-num shrinks the wipe; the
    bass-side sems (150+) are already range-cleared by the Tile teardown."""
    from concourse import bass_utils

    if getattr(bass_utils, "_max_sem_patched", False):
        return

    orig = bass_utils.run_command

    def run_command(argv, **kwargs):
        if argv and "walrus_driver" in str(argv[0]):
            argv = list(argv) + ["--max-sem-num=78"]
        return orig(argv, **kwargs)

    bass_utils.run_command = run_command
    bass_utils._max_sem_patched = True


_split_ctr = [0]


def _split_sync_waits(nc, maxw=1):
    for fn in nc.m.functions:
        for bb in fn.blocks:
            if not any(
                inst.sync_info is not None
                and inst.sync_info.on_wait
                and len(inst.sync_info.on_wait) > maxw
                for inst in bb.instructions
            ):
                continue
            new_list = []
            for inst in bb.instructions:
                si = inst.sync_info
                if si is not None and si.on_wait and len(si.on_wait) > maxw:
                    waits = list(si.on_wait)
                    extra, keep = waits[:-maxw], waits[-maxw:]
                    for i in range(0, len(extra), maxw):
                        _split_ctr[0] += 1
                        nop = mybir.InstNoOp(
                            name=f"splitw-{_split_ctr[0]}", ins=[], outs=[]
                        )
                        nop.engine = inst.engine
                        nop.sync_info = mybir.SyncInfo(
                            on_wait=extra[i : i + maxw], on_update=[]
                        )
                        new_list.append(nop)
                    inst.sync_info = mybir.SyncInfo(
                        on_wait=keep, on_update=list(si.on_update)
                    )
                new_list.append(inst)
            try:
                bb.instructions = new_list
            except Exception:
                bb.instructions.clear()
                bb.instructions.extend(new_list)


def _build():
    _patch_tile_drain()
    nc = bass.Bass("TRN2", target_bir_lowering=False, debug=False)

    # xs carries the two 1x1-conv weight panels in its first 128 columns,
    # then the x slab in CONSUMPTION order: the convs for partition-half rh
    # read slab rows 24rh+r, so the blocks are [h0 rows 0-14 | h1 rows
    # 24-38 | h0 rows 15-29 | h1 rows 39-53] (the 6 halo rows 24-29 are
    # stored twice).  DMA chunk A = weights+blocks 1-2 gates the first
    # half of the convs; chunk B = blocks 3-4 the rest.
    XB = 128          # weight-panel columns before the x slab
    NBR = 15          # slab rows per block
    XSW = XB + 4 * NBR * W
    xs_ext = nc.dram_tensor("xs", [128, XSW], FP16, kind="ExternalInput").ap()
    va_ext = nc.dram_tensor("vbufA", [128, SLAB, WPAD], BF16, kind="ExternalInput").ap()
    vb_ext = nc.dram_tensor("vbufB", [128, SLAB, WPAD], BF16, kind="ExternalInput").ap()
    rel_ext = nc.dram_tensor("relcols", [128, 8], F32, kind="ExternalInput").ap()
    id_ext = nc.dram_tensor("ident", [128, 128], BF16, kind="ExternalInput").ap()
    out_ext = nc.dram_tensor("out", [128, RH * W], F32, kind="ExternalOutput").ap()

    from contextlib import ExitStack

    with tile.TileContext(nc) as tc, ExitStack() as ctx:
        consts = ctx.enter_context(tc.tile_pool(name="consts", bufs=1))
        main = ctx.enter_context(tc.tile_pool(name="main", bufs=1))

        ident = consts.tile([128, 128], BF16)
        relc = consts.tile([128, 8], F32)
        vbufA = main.tile([128, SLAB, WPAD], BF16)
        vbufB = main.tile([128, SLAB, WPAD], BF16)
        q_sbs = [main.tile([128, FD], FP16, name=f"q{e}") for e in range(EP)]
        planesA = main.tile([128, K, SLAB, PW], FP16)
        planesB = main.tile([128, K, SLAB, PW], FP16)

        # ---- preamble: q/k 1x1 convs into the (channel, row-half) layout
        # xs lives in the persistent pool: reusing its space for the loop
        # pools would add a WAR edge from the first logit onto EVERY xs
        # consumer (measured +3us on the first TT).
        xs = main.tile([128, XSW], FP16, name="xs")
        wkT = xs[:, 0:64]
        wqT = xs[:, 64:128]
        scrpad = main.tile([128, 640], FP16, name="scrpad")

        def xoff(rh, r):
            """col offset of 'slab row 24rh+r' for partition-half rh."""
            blk = (1 if rh else 0) if r < NBR else (3 if rh else 2)
            return XB + (blk * NBR + (r - NBR if r >= NBR else r)) * W

        nc.gpsimd.memset(scrpad, 0.0)
        # chunk A (weights + conv-first-half rows), then B, then bulk vA on
        # the sync ring -- strictly in need-order so neither the scheduler's
        # DMA model nor HBM bandwidth can reorder the landings; vbufB rides
        # the otherwise-idle gpsimd ring.
        XMID = XB + 2 * NBR * W
        nc.sync.dma_start(out=xs[:, 0:XMID], in_=xs_ext[:, 0:XMID])
        nc.sync.dma_start(out=xs[:, XMID:XSW], in_=xs_ext[:, XMID:XSW])
        nc.sync.dma_start(out=vbufA, in_=va_ext)
        nc.sync.dma_start(out=vbufB, in_=vb_ext)
        # dummy ACT on scratch zeros (no DMA dep): hoists the ~1.3us
        # ACT_TABLE_LOAD off the critical path; then the two tiny const
        # DMAs ride the scalar ring.
        scr = consts.tile([128, 1], F32)
        nc.scalar.activation(
            out=scr, in_=scrpad[:, 0:1],
            func=mybir.ActivationFunctionType.Exp, bias=0.0, scale=1.0,
        )
        nc.scalar.dma_start(out=relc, in_=rel_ext)
        nc.scalar.dma_start(out=ident, in_=id_ext)

        NPRE = 6
        KW = SLAB * W // NPRE  # 480 = 5 rows
        QW = RH * W // NPRE    # 384 = 4 rows
        # q PSUM pool outlives the k pool: waves 1-2 are emitted INSIDE the
        # main loop (after the first logits) so their PSUM->SBUF casts don't
        # wedge ahead of the first logit TT in the DVE queue.  2 q banks +
        # 2x3 nd banks = 8 once the k pool's 6 are freed.
        ps_preQ = ctx.enter_context(tc.tile_pool(name="ps_preQ", bufs=1, space="PSUM"))
        q_ps = ps_preQ.tile([128, 2, 512], F32, tag="qps")

        def q_wave(w):
            """q conv wave for epoch w; the PSUM->SBUF cast rides the
            DVE (ramp slack) so the q chain never serializes on ACT."""
            for j in range(2):
                m = 2 * w + j
                for rh in range(2):
                    off = xoff(rh, 3 + 4 * m)
                    nc.tensor.matmul(
                        q_ps[64 * rh : 64 * rh + 64, j, 0:QW],
                        wqT,
                        xs[:, off : off + QW],
                        start=True, stop=True,
                    )
            nc.vector.tensor_copy(
                q_sbs[w].rearrange("p (c w) -> p c w", c=2),
                q_ps[:, :, 0:QW],
            )

        with tc.tile_pool(name="ps_pre", bufs=1, space="PSUM") as ps_pre:
            # Tile deps are TILE-granular: one k tile would make the first
            # plane0 ACT wait on the LAST k matmul (gated by DMA chunk B),
            # so the two conv halves get separate PSUM tiles.
            k_psA = ps_pre.tile([128, 3, 512], F32, tag="preA")
            k_psB = ps_pre.tile([128, 3, 512], F32, tag="preB")
            # PE warmup on scratch zeros: ~3.4us of sustained matmuls trips
            # the HAM activity monitor to 2.4GHz before the real convs.
            # Real k MMs overwrite via start=True (cols 480:512 of bank 0
            # are never read).
            for _ in range(8):
                nc.tensor.matmul(
                    k_psA[:, 0, 0:512],
                    scrpad[:, 0:128],
                    scrpad[:, 128:640],
                    start=True, stop=True,
                )

            def k_convs(c0, c1):
                for c in range(c0, c1):
                    kp = k_psA if c < 3 else k_psB
                    for rh in range(2):
                        off = xoff(rh, 5 * c)
                        nc.tensor.matmul(
                            kp[64 * rh : 64 * rh + 64, c % 3, 0:KW],
                            wkT,
                            xs[:, off : off + KW],
                            start=True, stop=True,
                        )

            # PE order: A-gated work first (k c0-2, q wave 0), then B-gated.
            # Waves 1-2 are emitted inside the main loop.
            k_convs(0, 3)
            q_wave(0)
            k_convs(3, NPRE)

            # plane 0 (A and B copies) straight from PSUM with the m=0
            # rel bias; chunk c covers slab rows 5c..5c+4.  Split in
            # row-halves: the first logit only reads plane rows 0..13.
            k_srcA = k_psA[:, :, 0:KW].rearrange("p c (r w) -> p c r w", r=5)
            k_srcB = k_psB[:, :, 0:KW].rearrange("p c (r w) -> p c r w", r=5)
            p0A = planesA[:, 0, :, 3 : 3 + W].rearrange(
                "p (c r) w -> p c r w", c=NPRE
            )
            p0B = planesB[:, 0, :, 2 : 2 + W].rearrange(
                "p (c r) w -> p c r w", c=NPRE
            )
            for cs, src in (((0, 3), k_srcA), ((3, NPRE), k_srcB)):
                for dst in (p0A, p0B):
                    nc.scalar.activation(
                        out=dst[:, cs[0] : cs[1]], in_=src,
                        func=mybir.ActivationFunctionType.Identity,
                        bias=relc[:, 0:1], scale=1.0,
                    )

        def strips(planes, ranges):
            """pad strips for plane 0: value = rel bias (k==0 in the pad).
            (ScalarE copy with a stride-0 src was tried: device INTERNAL
            error -- keep these on the DVE.)"""
            zt = relc[:, 0:1]
            for c0, c1 in ranges:
                dst = planes[:, 0, :, c0:c1]
                src = bass.AP(
                    zt.tensor, zt.offset, [zt.ap[0], [0, SLAB], [0, c1 - c0]]
                )
                nc.vector.tensor_copy(dst, src)

        def chain(planes, r0=0, r1=SLAB, m0=1, m1=K):
            """planes 1..6 = plane0 + (rel[m]-rel[0]), full width incl pads.
            Plane m is only ever read at rows [m, m+24), so clip per m."""
            for m in range(m0, m1):
                a, b = max(r0, m), min(r1, m + RH)
                if a >= b:
                    continue
                nc.vector.tensor_scalar(
                    out=planes[:, m, a:b], in0=planes[:, 0, a:b],
                    scalar1=relc[:, m : m + 1], scalar2=None,
                    op0=mybir.AluOpType.add,
                )

        strips(planesA, [(0, 3), (99, 102)])
        # chain for the FIRST logit in two pieces matching its dR-split:
        # planes 1-3 rows [m,11) gate TT(0,0)a; planes 4-6 rows [m,14)
        # gate TT(0,0)b.  (The remainders ride inside the loop.)
        chain(planesA, 0, 11, 1, 4)

        # ---- main loop: 3 epochs x 7 dC groups, software-pipelined by one
        # group so ScalarE's EXP[g] runs while group g+1's logits build.
        lgp = ctx.enter_context(tc.tile_pool(name="lgp", bufs=2))
        ep_pool = ctx.enter_context(tc.tile_pool(name="ep", bufs=3))
        outp = ctx.enter_context(tc.tile_pool(name="outp", bufs=1))
        ps_loop = ctx.enter_context(tc.tile_pool(name="ps_loop", bufs=2, space="PSUM"))

        nd_tiles = {}

        def phase_logit(e, dC, dR_splits=None):
            """logit[dR] = q * plane_dR window (one TT for all 7 dR, or one
            per dR-span for the first group so work starts before the full
            plane chain is built)."""
            P = dC & 1
            dC2 = dC - P
            planes = planesB if P else planesA
            lg = lgp.tile([128, K, FD], FP16, tag="lg")
            pl = planes[:, 0]
            qs = q_sbs[e][:, 0:FD]
            for m0, m1 in dR_splits or [(0, K)]:
                in1 = bass.AP(
                    pl.tensor,
                    pl.offset + m0 * (PLSTRIDE + PW) + EP_ROWS * e * PW + dC2,
                    [pl.ap[0], [PLSTRIDE + PW, m1 - m0], [PW, EP_ROWS], [1, W]],
                )
                q_bc = bass.AP(
                    qs.tensor, qs.offset,
                    [qs.ap[0], [0, m1 - m0], [W, EP_ROWS], [1, W]],
                )
                nc.vector.tensor_tensor(
                    out=lg[:, m0:m1].rearrange("p k (r w) -> p k r w", r=EP_ROWS),
                    in0=q_bc, in1=in1, op=mybir.AluOpType.mult,
                )
                if dR_splits and m1 < K:
                    # build the next span's plane rows before its TT
                    chain(planesA, 0, 14, m1, K)
            return lg

        # NOTE: offloading e*v taps to GpSimd was tried and REGRESSED badly:
        # GpSimd shares SBUF ports with the DVE, so concurrent GpSimd tensor
        # ops knock the DVE's tensor_tensor out of 2x mode (~2x slowdown).
        GP_TAPS = 0  # dR taps of the e*v product offloaded to GpSimd

        def phase_rest(e, dC, lg, split=False):
            """exp, e*v, and the accumulate matmuls for group (e, dC). With
            split=True (the very last group) run in two dR-halves so the
            tail matmuls/normalize start earlier."""
            P = dC & 1
            dC2 = dC - P
            vbuf = vbufB if P else vbufA
            ept = ep_pool.tile([128, K, 2, FD], BF16, tag="ept")
            if dC == 0:
                # one PSUM tile per 256-px chunk: deps are tile-granular,
                # so chunk c's normalize must not wait on other chunks' MMs
                nd_tiles[e] = [
                    ps_loop.tile([128, 512], F32, tag=f"nd{c}", name=f"nd{e}_{c}")
                    for c in range(len(CHUNKS))
                ]
            nd_ps = nd_tiles[e]
            rowstep = vbuf.ap[1][0]

            def prod(i0, i1, eng):
                vbase = vbuf[
                    :, EP_ROWS * e + i0 : EP_ROWS * e + i0 + 1, dC2 : dC2 + W
                ]
                v_ov = bass.AP(
                    vbase.tensor, vbase.offset,
                    [vbase.ap[0], [rowstep, i1 - i0], [rowstep, EP_ROWS], [1, W]],
                )
                eng.tensor_tensor(
                    out=ept[:, i0:i1, 0, :].rearrange(
                        "p k (r w) -> p k r w", r=EP_ROWS
                    ),
                    in0=ept[:, i0:i1, 1, :].rearrange(
                        "p k (r w) -> p k r w", r=EP_ROWS
                    ),
                    in1=v_ov, op=mybir.AluOpType.mult,
                )

            if split:
                # tail groups: per-tap prods with the tap's matmuls emitted
                # right behind, so the PE streams close behind the DVE
                # instead of draining 21 matmuls after the last prod.  The
                # ept tile's last-writer dep then points at THIS tap's prod.
                for h0, h1 in ((0, 4), (4, K)):
                    nc.scalar.activation(
                        out=ept[:, h0:h1, 1, :], in_=lg[:, h0:h1, :],
                        func=mybir.ActivationFunctionType.Exp,
                        bias=0.0, scale=1.0,
                    )
                    for i in range(h0, h1):
                        prod(i, i + 1, nc.vector)
                        for c, (px0, cw) in enumerate(CHUNKS):
                            nc.tensor.matmul(
                                nd_ps[c][:, 0 : 2 * cw], ident,
                                ept[:, i, :, px0 : px0 + cw],
                                start=(dC == 0 and i == 0),
                                stop=(dC == K - 1 and i == K - 1),
                            )
                return
            nc.scalar.activation(
                out=ept[:, :, 1, :], in_=lg,
                func=mybir.ActivationFunctionType.Exp, bias=0.0, scale=1.0,
            )
            prod(0, K, nc.vector)
            for i in range(K):
                for c, (px0, cw) in enumerate(CHUNKS):
                    nc.tensor.matmul(
                        nd_ps[c][:, 0 : 2 * cw], ident,
                        ept[:, i, :, px0 : px0 + cw],
                        start=(dC == 0 and i == 0),
                        stop=(dC == K - 1 and i == K - 1),
                    )

        def normalize(e, per_chunk=False):
            """out = num * exp(-ln(den)); [num|den] interleaved 256s.
            Always per 256-px chunk: each chunk's ln only waits ITS OWN
            accumulator tile's stop-matmul, so normalize+DMA overlap the
            remaining chunks' matmuls."""
            nd_ps = nd_tiles[e]
            NC = len(CHUNKS)
            lnden = outp.tile([128, FD], F32, tag="lnden")
            recip = outp.tile([128, FD], F32, tag="recip")
            out_sb = outp.tile([128, FD], F32, tag="out_sb")
            for c in range(NC):
                nc.scalar.activation(
                    out=lnden[:, c * 256 : (c + 1) * 256],
                    in_=nd_ps[c][:, 256:512],
                    func=mybir.ActivationFunctionType.Ln, bias=0.0, scale=1.0,
                )
                nc.scalar.activation(
                    out=recip[:, c * 256 : (c + 1) * 256],
                    in_=lnden[:, c * 256 : (c + 1) * 256],
                    func=mybir.ActivationFunctionType.Exp, bias=0.0, scale=-1.0,
                )
                nc.vector.tensor_tensor(
                    out=out_sb[:, c * 256 : (c + 1) * 256],
                    in0=nd_ps[c][:, 0:256],
                    in1=recip[:, c * 256 : (c + 1) * 256],
                    op=mybir.AluOpType.mult,
                )
                nc.sync.dma_start(
                    out=out_ext[:, e * FD + c * 256 : e * FD + (c + 1) * 256],
                    in_=out_sb[:, c * 256 : (c + 1) * 256],
                )

        groups = [(e, dC) for e in range(EP) for dC in range(K)]
        pending = None  # (e, dC, lg) whose exp/prod/MMs are not yet emitted
        for e, dC in groups:
            lg = phase_logit(
                e, dC, dR_splits=[(0, 4), (4, K)] if (e, dC) == (0, 0) else None
            )
            if (e, dC) == (0, 0):
                # B planes are first needed by group (0,1); building them
                # here overlaps the chain with group (0,0)'s exp.
                strips(planesB, [(0, 2), (98, 102)])
                chain(planesB, 0, 14)
                q_wave(1)
            elif (e, dC) == (0, 1):
                # A-plane remainders (needed from epoch 1 on)
                chain(planesA, 11, SLAB, 1, 4)
                chain(planesA, 14, SLAB, 4, K)
                q_wave(2)
            elif (e, dC) == (0, 2):
                chain(planesB, 14, SLAB)
            if pending is not None:
                pe, pc, plg = pending
                phase_rest(pe, pc, plg, split=(pe == EP - 1 and pc >= K - 4))
                # normalize(e-1) is deferred until after group (e,0)'s rest:
                # its ln/exp would otherwise sit in the ACT queue AHEAD of
                # exp(e,0) and stall the next epoch's first e*v (~0.8us).
                if pc == 0 and pe > 0:
                    normalize(pe - 1)
            pending = (e, dC, lg)
        pe, pc, plg = pending
        phase_rest(pe, pc, plg, split=True)
        normalize(pe, per_chunk=True)

    _split_sync_waits(nc)
    return nc


def _host_prep(x, v, w_q, w_k, rel_h, rel_w):
    """Build the 8 per-core input maps (numpy only)."""
    x = np.asarray(x, np.float32)
    v = np.asarray(v, np.float32)
    w_q = np.asarray(w_q, np.float32)
    w_k = np.asarray(w_k, np.float32)
    rel_h = np.asarray(rel_h, np.float32).reshape(64, K)   # [c, i]
    rel_w = np.asarray(rel_w, np.float32).reshape(64, K)   # [c, j]

    ident = np.eye(128, dtype=np.float32).astype(ml_dtypes.bfloat16)

    in_maps = []
    for ci in range(NCORES):
        b, rest = divmod(ci, 4)
        half, t = divmod(rest, 2)
        ch0 = 64 * half
        if half == 0:
            xf = x[b]                          # [128, R=h, C=w]
            vf = v[b, ch0 : ch0 + 64]
            relv = rel_h                       # [c, m] with m = dR
        else:
            xf = np.ascontiguousarray(x[b].transpose(0, 2, 1))   # R=w, C=h
            vf = np.ascontiguousarray(v[b, ch0 : ch0 + 64].transpose(0, 2, 1))
            relv = rel_w

        R0 = RT * t
        # x slab: stored rows R0-3 .. R0+50, zero beyond the image
        xs = np.zeros((128, XSLAB, W), np.float32)
        glo, ghi = max(0, R0 - PAD), min(96, R0 + RT + PAD)
        xs[:, glo - (R0 - PAD) : ghi - (R0 - PAD), :] = xf[:, glo:ghi, :]

        # v family buffer: partition p = c + 64*rh
        vs = np.zeros((2, 64, SLAB, WPAD), np.float32)
        for rh in range(2):
            r0 = R0 + RH * rh
            lo, hi = max(0, r0 - PAD), min(96, r0 + RH + PAD)
            vs[rh, :, lo - (r0 - PAD) : hi - (r0 - PAD), PAD : PAD + W] = (
                vf[:, lo:hi, :]
            )
        vbufA = vs.reshape(128, SLAB, WPAD)
        vbufB = np.zeros_like(vbufA)
        vbufB[:, :, : WPAD - 1] = vbufA[:, :, 1:]

        relc = np.zeros((128, 8), np.float32)
        rv = np.concatenate([relv, relv], axis=0)          # [128, 7], p=c+64rh
        relc[:, 0] = rv[:, 0]
        relc[:, 1:K] = rv[:, 1:K] - rv[:, 0:1]

        # blocks: [weights | h0 slab rows 0-14 | h1 rows 24-38 |
        #          h0 rows 15-29 | h1 rows 39-53]
        xs_pack = np.empty((128, 128 + 4 * 15 * W), np.float16)
        xs_pack[:, 0:64] = w_k[ch0 : ch0 + 64].T.astype(np.float16)
        xs_pack[:, 64:128] = w_q[ch0 : ch0 + 64].T.astype(np.float16)
        xs16 = xs.astype(np.float16)
        for b, r0 in enumerate((0, 24, 15, 39)):
            xs_pack[:, 128 + b * 15 * W : 128 + (b + 1) * 15 * W] = xs16[
                :, r0 : r0 + 15, :
            ].reshape(128, 15 * W)
        in_maps.append(
            {
                "xs": np.ascontiguousarray(xs_pack),
                "vbufA": np.ascontiguousarray(vbufA.astype(ml_dtypes.bfloat16)),
                "vbufB": np.ascontiguousarray(vbufB.astype(ml_dtypes.bfloat16)),
                "relcols": relc,
                "ident": ident,
            }
        )
    return in_maps


def kernel(x, v, w_q, w_k, rel_h, rel_w, trace=False, tmpdir=None):
    from concourse.bass_utils import run_bass_kernel_spmd

    if "nc" not in _cache:
        _cache["nc"] = _build()
    nc = _cache["nc"]
    in_maps = _host_prep(x, v, w_q, w_k, rel_h, rel_w)
    res = run_bass_kernel_spmd(
        nc, in_maps, list(range(NCORES)), trace=trace, tmpdir=tmpdir
    )
    out = np.zeros((B, COUT, H, W), np.float32)
    for ci in range(NCORES):
        b, rest = divmod(ci, 4)
        half, t = divmod(rest, 2)
        ch0, R0 = 64 * half, RT * t
        a = res.results[ci]["out"].reshape(2, 64, EP, EP_ROWS, W)
        lines = a.transpose(1, 0, 2, 3, 4).reshape(64, RT, W)  # [c, line, C]
        if half == 0:
            out[b, ch0 : ch0 + 64, R0 : R0 + RT, :] = lines
        else:
            out[b, ch0 : ch0 + 64, :, R0 : R0 + RT] = lines.transpose(0, 2, 1)
    kernel.last_exec_time_ns = res.exec_time_ns
    kernel.last_results = res
    return out



# revision 46
# speedup vs baseline: 1.0050x; 1.0015x over previous
"""Trainium2 Bass kernel for 7x7 local (sparse) attention, SPMD over 8 NeuronCores.

Math (per channel c, pixel p):
    q = w_q @ x, k = w_k @ x            (1x1 convs)
    logit[c,p,(i,j)] = q[c,p] * (kpad[c, p+(i,j)] + rel[c,(i,j)])
    out[c,p] = sum_k softmax_k(logit) * vpad[c, p+(i,j)]
where rel[c,(i,j)] = rel_h[c,i] for c<64, rel_w[c-64,j] for c>=64, and
kpad/vpad are zero-padded by 3 (padded taps contribute exp(q*rel) to the
softmax denominator -- matching the reference exactly).

Sharding: 8 cores = 2 batches x 2 CHANNEL halves x 2 spatial 48-line tiles.
High-half cores store all spatial data TRANSPOSED (W-major), so that the rel
bias always attaches to the stored-ROW tap (rel_h[c,i] for low cores, rel_w
[c,j] for high ones) and one NEFF serves all 8 cores.  Within a core the
128 partitions hold (64 channels x 2 row-halves of 24 lines each).

This layout lets the per-tap "krel = k-window + rel" pass (49 taps worth of
adds per pixel) collapse into SEVEN pre-biased k planes built once per core
(plane_m = k + rel[c,m]; window overlap gives the 7x redundancy win), plus a
1-col-shifted B copy of each for the odd column taps (DVE 2x mode needs
4B-aligned windows).  The logit tensor_tensor then reads plane windows
directly:  logit[c,(dR,dC),p] = q[c,p] * plane_dR[c, p + (dR,dC)].

Per-core device pipeline (21 groups = 3 row-epochs x 7 dC):
  DVE: one batched tensor_tensor per group for logits (q broadcast over the
       7 dR via a stride-0 AP reading the 7 planes via a strided window AP)
       and one for e*v (overlapping-row v AP), both fp16/bf16 2x mode.
  ACT: one exp per group (7 dR batched), and the final recip = exp(-ln(den)).
  PE:  q/k 1x1 convs (64-wide weights, written into both partition halves),
       then one identity-matmul per (dR, 256-px chunk) accumulating
       [num | den] jointly into 5 PSUM banks per epoch.
The emission is software-pipelined by one group so ScalarE's exp[g] overlaps
group g+1's logit TT.  Output epoch = 8 lines x 96 cols: [num|den] for 768 px
= 3 PSUM banks, so two epochs' accumulators coexist and the epoch boundary
(normalize of e vs first matmuls of e+1) fully overlaps.

Edge scheduling (the steady state is DVE-bound and already dense, so the
wins are at the edges; hard-won notes):
 - Per-DMA completion latency here is ~3-4us and transfers serialize per
   ring, so xs is layout-reordered into conv-consumption blocks (weights
   first, halo rows duplicated) and chunked over the sync ring in strict
   need-order; bulk v queues behind it.  The scalar ring stays DMA-light
   so the ACT table load + plane0 build start immediately.
 - Tile-framework deps on PSUM are effectively TILE-granular: k/q/nd PSUM
   accumulators are split into per-consumer tiles, else a reader waits the
   tile's LAST writer (first plane0 ACT waited on the last k-conv; the
   per-chunk normalize waited on ALL 21 matmuls).
 - 8 warmup matmuls on scratch zeros trip the HAM activity monitor so the
   real convs run at 2.4GHz.
 - The last 4 groups run a fine per-tap exp/prod/matmul pipeline and the
   final epoch normalizes + DMAs per 256-px chunk, cutting the end-of-
   kernel PE drain; each epoch's normalize is emitted one group late so
   its ln/exp don't delay the next epoch's first e*v.
 - ~13us is fixed runtime overhead: engine init barriers at entry plus a
   full 264-semaphore per-engine wipe the runtime appends at the end.

fp16 is used for the whole logit chain (q, planes, logit): bf16's 8-bit
mantissa on |logit|<=60 costs ~4% absmax error; fp16 keeps it ~1%.
e/prod are bf16 (need range: e up to exp(60)).
"""

import sys

import numpy as np
import ml_dtypes

sys.path.insert(0, "/opt/trn_rl_repo")

import concourse.bass as bass  # noqa: E402
import concourse.tile as tile  # noqa: E402
from concourse import mybir  # noqa: E402
from concourse.vector_clock import ScopedClock  # noqa: E402

F32 = mybir.dt.float32
BF16 = mybir.dt.bfloat16
FP16 = mybir.dt.float16

B, CIN, COUT, H, W, K, PAD = 2, 128, 128, 96, 96, 7, 3
NCORES = 8
RT = 48          # stored-R lines per core
RH = 24          # lines per partition half
SLAB = RH + 2 * PAD   # 30 rows incl halo per half
XSLAB = RT + 2 * PAD  # 54 rows of x slab
PW = 102         # padded plane width (3 + 96 + 3)
WPAD = 104       # v buffer width (96 + 6 pad + 2 spare)
EP_ROWS = 8      # lines per PSUM epoch
EP = RH // EP_ROWS    # 3 epochs
FD = EP_ROWS * W      # 768
PLSTRIDE = SLAB * PW  # 3060 elements per plane
# PSUM chunking: each bank holds [num(256 px) | den(same px)] so one matmul
# accumulates both. 768 px = 3 chunks of 256 -> 3 banks, 3 MMs; two epochs'
# accumulators coexist (3+3 of 8 banks) so the epoch boundary overlaps.
CHUNKS = [(0, 256), (256, 256), (512, 256)]

_cache = {}


def _patch_tile_drain():
    """walrus in this container allows only one sync-wait per instruction;
    split excess waits onto NoOps."""
    if getattr(tile, "_drain_patched", False):
        return

    def _drain_and_barrier(self, tick_clock, wait_clock):
        drain_inst = self.nc.sync.drain()
        wait_clock.add_sem_waits(
            drain_inst.ins, ScopedClock({None: tick_clock.global_clock})
        )
        si = drain_inst.ins.sync_info
        if si is not None and si.on_wait and len(si.on_wait) > 1:
            waits = list(si.on_wait)
            drain_inst.ins.sync_info = mybir.SyncInfo(
                on_wait=waits[:1], on_update=list(si.on_update)
            )
            for w in waits[1:]:
                nop_inst = self.nc.sync.nop()
                nop_inst.ins.sync_info = mybir.SyncInfo(on_wait=[w], on_update=[])
        self.nc.all_engine_barrier()
        assert self.sems is not None
        popped = self.nc._tile_sem_poison_stack.pop()
        assert popped is self._sem_poison
        self.nc.clear_and_free_semaphores(list(self.sems.allocated().values()))
        self.nc.all_engine_barrier()

    tile.TileContext._drain_and_barrier = _drain_and_barrier
    tile._drain_patched = True


_split_ctr = [0]


def _split_sync_waits(nc, maxw=1):
    for fn in nc.m.functions:
        for bb in fn.blocks:
            if not any(
                inst.sync_info is not None
                and inst.sync_info.on_wait
                and len(inst.sync_info.on_wait) > maxw
                for inst in bb.instructions
            ):
                continue
            new_list = []
            for inst in bb.instructions:
                si = inst.sync_info
                if si is not None and si.on_wait and len(si.on_wait) > maxw:
                    waits = list(si.on_wait)
                    extra, keep = waits[:-maxw], waits[-maxw:]
                    for i in range(0, len(extra), maxw):
                        _split_ctr[0] += 1
                        nop = mybir.InstNoOp(
                            name=f"splitw-{_split_ctr[0]}", ins=[], outs=[]
                        )
                        nop.engine = inst.engine
                        nop.sync_info = mybir.SyncInfo(
                            on_wait=extra[i : i + maxw], on_update=[]
                        )
                        new_list.append(nop)
                    inst.sync_info = mybir.SyncInfo(
                        on_wait=keep, on_update=list(si.on_update)
                    )
                new_list.append(inst)
            try:
                bb.instructions = new_list
            except Exception:
                bb.instructions.clear()
                bb.instructions.extend(new_list)


def _build():
    _patch_tile_drain()
    nc = bass.Bass("TRN2", target_bir_lowering=False, debug=False)

    # xs carries the two 1x1-conv weight panels in its first 128 columns,
    # then the x slab in CONSUMPTION order: the convs for partition-half rh
    # read slab rows 24rh+r, so the blocks are [h0 rows 0-14 | h1 rows
    # 24-38 | h0 rows 15-29 | h1 rows 39-53] (the 6 halo rows 24-29 are
    # stored twice).  DMA chunk A = weights+blocks 1-2 gates the first
    # half of the convs; chunk B = blocks 3-4 the rest.
    XB = 128          # weight-panel columns before the x slab
    NBR = 15          # slab rows per block
    XSW = XB + 4 * NBR * W
    xs_ext = nc.dram_tensor("xs", [128, XSW], FP16, kind="ExternalInput").ap()
    va_ext = nc.dram_tensor("vbufA", [128, SLAB, WPAD], BF16, kind="ExternalInput").ap()
    vb_ext = nc.dram_tensor("vbufB", [128, SLAB, WPAD], BF16, kind="ExternalInput").ap()
    rel_ext = nc.dram_tensor("relcols", [128, 8], F32, kind="ExternalInput").ap()
    id_ext = nc.dram_tensor("ident", [128, 128], BF16, kind="ExternalInput").ap()
    out_ext = nc.dram_tensor("out", [128, RH * W], F32, kind="ExternalOutput").ap()

    from contextlib import ExitStack

    with tile.TileContext(nc) as tc, ExitStack() as ctx:
        consts = ctx.enter_context(tc.tile_pool(name="consts", bufs=1))
        main = ctx.enter_context(tc.tile_pool(name="main", bufs=1))

        ident = consts.tile([128, 128], BF16)
        relc = consts.tile([128, 8], F32)
        vbufA = main.tile([128, SLAB, WPAD], BF16)
        vbufB = main.tile([128, SLAB, WPAD], BF16)
        q_sbs = [main.tile([128, FD], FP16, name=f"q{e}") for e in range(EP)]
        planesA = main.tile([128, K, SLAB, PW], FP16)
        planesB = main.tile([128, K, SLAB, PW], FP16)

        # ---- preamble: q/k 1x1 convs into the (channel, row-half) layout
        # xs lives in the persistent pool: reusing its space for the loop
        # pools would add a WAR edge from the first logit onto EVERY xs
        # consumer (measured +3us on the first TT).
        xs = main.tile([128, XSW], FP16, name="xs")
        wkT = xs[:, 0:64]
        wqT = xs[:, 64:128]
        scrpad = main.tile([128, 640], FP16, name="scrpad")

        def xoff(rh, r):
            """col offset of 'slab row 24rh+r' for partition-half rh."""
            blk = (1 if rh else 0) if r < NBR else (3 if rh else 2)
            return XB + (blk * NBR + (r - NBR if r >= NBR else r)) * W

        nc.gpsimd.memset(scrpad, 0.0)
        # chunk A (weights + conv-first-half rows), then B, then bulk vA on
        # the sync ring -- strictly in need-order so neither the scheduler's
        # DMA model nor HBM bandwidth can reorder the landings; vbufB rides
        # the otherwise-idle gpsimd ring.
        XMID = XB + 2 * NBR * W
        nc.sync.dma_start(out=xs[:, 0:XMID], in_=xs_ext[:, 0:XMID])
        nc.sync.dma_start(out=xs[:, XMID:XSW], in_=xs_ext[:, XMID:XSW])
        nc.sync.dma_start(out=vbufA, in_=va_ext)
        nc.sync.dma_start(out=vbufB, in_=vb_ext)
        # dummy ACT on scratch zeros (no DMA dep): hoists the ~1.3us
        # ACT_TABLE_LOAD off the critical path; then the two tiny const
        # DMAs ride the scalar ring.
        scr = consts.tile([128, 1], F32)
        nc.scalar.activation(
            out=scr, in_=scrpad[:, 0:1],
            func=mybir.ActivationFunctionType.Exp, bias=0.0, scale=1.0,
        )
        nc.scalar.dma_start(out=relc, in_=rel_ext)
        nc.scalar.dma_start(out=ident, in_=id_ext)

        NPRE = 6
        KW = SLAB * W // NPRE  # 480 = 5 rows
        QW = RH * W // NPRE    # 384 = 4 rows
        # q PSUM pool outlives the k pool: waves 1-2 are emitted INSIDE the
        # main loop (after the first logits) so their PSUM->SBUF casts don't
        # wedge ahead of the first logit TT in the DVE queue.  2 q banks +
        # 2x3 nd banks = 8 once the k pool's 6 are freed.
        ps_preQ = ctx.enter_context(tc.tile_pool(name="ps_preQ", bufs=1, space="PSUM"))
        q_ps = ps_preQ.tile([128, 2, 512], F32, tag="qps")

        def q_wave(w):
            """q conv wave for epoch w; the PSUM->SBUF cast rides the
            DVE (ramp slack) so the q chain never serializes on ACT."""
            for j in range(2):
                m = 2 * w + j
                for rh in range(2):
                    off = xoff(rh, 3 + 4 * m)
                    nc.tensor.matmul(
                        q_ps[64 * rh : 64 * rh + 64, j, 0:QW],
                        wqT,
                        xs[:, off : off + QW],
                        start=True, stop=True,
                    )
            nc.vector.tensor_copy(
                q_sbs[w].rearrange("p (c w) -> p c w", c=2),
                q_ps[:, :, 0:QW],
            )

        with tc.tile_pool(name="ps_pre", bufs=1, space="PSUM") as ps_pre:
            # Tile deps are TILE-granular: one k tile would make the first
            # plane0 ACT wait on the LAST k matmul (gated by DMA chunk B),
            # so the two conv halves get separate PSUM tiles.
            k_psA = ps_pre.tile([128, 3, 512], F32, tag="preA")
            k_psB = ps_pre.tile([128, 3, 512], F32, tag="preB")
            # PE warmup on scratch zeros: ~3.4us of sustained matmuls trips
            # the HAM activity monitor to 2.4GHz before the real convs.
            # Real k MMs overwrite via start=True (cols 480:512 of bank 0
            # are never read).
            for _ in range(8):
                nc.tensor.matmul(
                    k_psA[:, 0, 0:512],
                    scrpad[:, 0:128],
                    scrpad[:, 128:640],
                    start=True, stop=True,
                )

            def k_convs(c0, c1):
                for c in range(c0, c1):
                    kp = k_psA if c < 3 else k_psB
                    for rh in range(2):
                        off = xoff(rh, 5 * c)
                        nc.tensor.matmul(
                            kp[64 * rh : 64 * rh + 64, c % 3, 0:KW],
                            wkT,
                            xs[:, off : off + KW],
                            start=True, stop=True,
                        )

            # PE order: A-gated work first (k c0-2, q wave 0), then B-gated.
            # Waves 1-2 are emitted inside the main loop.
            k_convs(0, 3)
            q_wave(0)
            k_convs(3, NPRE)

            # plane 0 (A and B copies) straight from PSUM with the m=0
            # rel bias; chunk c covers slab rows 5c..5c+4.  Split in
            # row-halves: the first logit only reads plane rows 0..13.
            k_srcA = k_psA[:, :, 0:KW].rearrange("p c (r w) -> p c r w", r=5)
            k_srcB = k_psB[:, :, 0:KW].rearrange("p c (r w) -> p c r w", r=5)
            p0A = planesA[:, 0, :, 3 : 3 + W].rearrange(
                "p (c r) w -> p c r w", c=NPRE
            )
            p0B = planesB[:, 0, :, 2 : 2 + W].rearrange(
                "p (c r) w -> p c r w", c=NPRE
            )
            for cs, src in (((0, 3), k_srcA), ((3, NPRE), k_srcB)):
                for dst in (p0A, p0B):
                    nc.scalar.activation(
                        out=dst[:, cs[0] : cs[1]], in_=src,
                        func=mybir.ActivationFunctionType.Identity,
                        bias=relc[:, 0:1], scale=1.0,
                    )

        def strips(planes, ranges):
            """pad strips for plane 0: value = rel bias (k==0 in the pad).
            (ScalarE copy with a stride-0 src was tried: device INTERNAL
            error -- keep these on the DVE.)"""
            zt = relc[:, 0:1]
            for c0, c1 in ranges:
                dst = planes[:, 0, :, c0:c1]
                src = bass.AP(
                    zt.tensor, zt.offset, [zt.ap[0], [0, SLAB], [0, c1 - c0]]
                )
                nc.vector.tensor_copy(dst, src)

        def chain(planes, r0=0, r1=SLAB, m0=1, m1=K):
            """planes 1..6 = plane0 + (rel[m]-rel[0]), full width incl pads.
            Plane m is only ever read at rows [m, m+24), so clip per m."""
            for m in range(m0, m1):
                a, b = max(r0, m), min(r1, m + RH)
                if a >= b:
                    continue
                nc.vector.tensor_scalar(
                    out=planes[:, m, a:b], in0=planes[:, 0, a:b],
                    scalar1=relc[:, m : m + 1], scalar2=None,
                    op0=mybir.AluOpType.add,
                )

        strips(planesA, [(0, 3), (99, 102)])
        # chain for the FIRST logit in two pieces matching its dR-split:
        # planes 1-3 rows [m,11) gate TT(0,0)a; planes 4-6 rows [m,14)
        # gate TT(0,0)b.  (The remainders ride inside the loop.)
        chain(planesA, 0, 11, 1, 4)

        # ---- main loop: 3 epochs x 7 dC groups, software-pipelined by one
        # group so ScalarE's EXP[g] runs while group g+1's logits build.
        lgp = ctx.enter_context(tc.tile_pool(name="lgp", bufs=2))
        ep_pool = ctx.enter_context(tc.tile_pool(name="ep", bufs=3))
        outp = ctx.enter_context(tc.tile_pool(name="outp", bufs=1))
        ps_loop = ctx.enter_context(tc.tile_pool(name="ps_loop", bufs=2, space="PSUM"))

        nd_tiles = {}

        def phase_logit(e, dC, dR_splits=None):
            """logit[dR] = q * plane_dR window (one TT for all 7 dR, or one
            per dR-span for the first group so work starts before the full
            plane chain is built)."""
            P = dC & 1
            dC2 = dC - P
            planes = planesB if P else planesA
            lg = lgp.tile([128, K, FD], FP16, tag="lg")
            pl = planes[:, 0]
            qs = q_sbs[e][:, 0:FD]
            for m0, m1 in dR_splits or [(0, K)]:
                in1 = bass.AP(
                    pl.tensor,
                    pl.offset + m0 * (PLSTRIDE + PW) + EP_ROWS * e * PW + dC2,
                    [pl.ap[0], [PLSTRIDE + PW, m1 - m0], [PW, EP_ROWS], [1, W]],
                )
                q_bc = bass.AP(
                    qs.tensor, qs.offset,
                    [qs.ap[0], [0, m1 - m0], [W, EP_ROWS], [1, W]],
                )
                nc.vector.tensor_tensor(
                    out=lg[:, m0:m1].rearrange("p k (r w) -> p k r w", r=EP_ROWS),
                    in0=q_bc, in1=in1, op=mybir.AluOpType.mult,
                )
                if dR_splits and m1 < K:
                    # build the next span's plane rows before its TT
                    chain(planesA, 0, 14, m1, K)
            return lg

        # NOTE: offloading e*v taps to GpSimd was tried and REGRESSED badly:
        # GpSimd shares SBUF ports with the DVE, so concurrent GpSimd tensor
        # ops knock the DVE's tensor_tensor out of 2x mode (~2x slowdown).
        GP_TAPS = 0  # dR taps of the e*v product offloaded to GpSimd

        def phase_rest(e, dC, lg, split=False):
            """exp, e*v, and the accumulate matmuls for group (e, dC). With
            split=True (the very last group) run in two dR-halves so the
            tail matmuls/normalize start earlier."""
            P = dC & 1
            dC2 = dC - P
            vbuf = vbufB if P else vbufA
            ept = ep_pool.tile([128, K, 2, FD], BF16, tag="ept")
            if dC == 0:
                # one PSUM tile per 256-px chunk: deps are tile-granular,
                # so chunk c's normalize must not wait on other chunks' MMs
                nd_tiles[e] = [
                    ps_loop.tile([128, 512], F32, tag=f"nd{c}", name=f"nd{e}_{c}")
                    for c in range(len(CHUNKS))
                ]
            nd_ps = nd_tiles[e]
            rowstep = vbuf.ap[1][0]

            def prod(i0, i1, eng):
                vbase = vbuf[
                    :, EP_ROWS * e + i0 : EP_ROWS * e + i0 + 1, dC2 : dC2 + W
                ]
                v_ov = bass.AP(
                    vbase.tensor, vbase.offset,
                    [vbase.ap[0], [rowstep, i1 - i0], [rowstep, EP_ROWS], [1, W]],
                )
                eng.tensor_tensor(
                    out=ept[:, i0:i1, 0, :].rearrange(
                        "p k (r w) -> p k r w", r=EP_ROWS
                    ),
                    in0=ept[:, i0:i1, 1, :].rearrange(
                        "p k (r w) -> p k r w", r=EP_ROWS
                    ),
                    in1=v_ov, op=mybir.AluOpType.mult,
                )

            if split:
                # tail groups: per-tap prods with the tap's matmuls emitted
                # right behind, so the PE streams close behind the DVE
                # instead of draining 21 matmuls after the last prod.  The
                # ept tile's last-writer dep then points at THIS tap's prod.
                for h0, h1 in ((0, 4), (4, K)):
                    nc.scalar.activation(
                        out=ept[:, h0:h1, 1, :], in_=lg[:, h0:h1, :],
                        func=mybir.ActivationFunctionType.Exp,
                        bias=0.0, scale=1.0,
                    )
                    for i in range(h0, h1):
                        prod(i, i + 1, nc.vector)
                        for c, (px0, cw) in enumerate(CHUNKS):
                            nc.tensor.matmul(
                                nd_ps[c][:, 0 : 2 * cw], ident,
                                ept[:, i, :, px0 : px0 + cw],
                                start=(dC == 0 and i == 0),
                                stop=(dC == K - 1 and i == K - 1),
                            )
                return
            nc.scalar.activation(
                out=ept[:, :, 1, :], in_=lg,
                func=mybir.ActivationFunctionType.Exp, bias=0.0, scale=1.0,
            )
            prod(0, K, nc.vector)
            for i in range(K):
                for c, (px0, cw) in enumerate(CHUNKS):
                    nc.tensor.matmul(
                        nd_ps[c][:, 0 : 2 * cw], ident,
                        ept[:, i, :, px0 : px0 + cw],
                        start=(dC == 0 and i == 0),
                        stop=(dC == K - 1 and i == K - 1),
                    )

        def normalize(e, per_chunk=False):
            """out = num * exp(-ln(den)); [num|den] interleaved 256s.
            Always per 256-px chunk: each chunk's ln only waits ITS OWN
            accumulator tile's stop-matmul, so normalize+DMA overlap the
            remaining chunks' matmuls."""
            nd_ps = nd_tiles[e]
            NC = len(CHUNKS)
            lnden = outp.tile([128, FD], F32, tag="lnden")
            recip = outp.tile([128, FD], F32, tag="recip")
            out_sb = outp.tile([128, FD], F32, tag="out_sb")
            for c in range(NC):
                nc.scalar.activation(
                    out=lnden[:, c * 256 : (c + 1) * 256],
                    in_=nd_ps[c][:, 256:512],
                    func=mybir.ActivationFunctionType.Ln, bias=0.0, scale=1.0,
                )
                nc.scalar.activation(
                    out=recip[:, c * 256 : (c + 1) * 256],
                    in_=lnden[:, c * 256 : (c + 1) * 256],
                    func=mybir.ActivationFunctionType.Exp, bias=0.0, scale=-1.0,
                )
                nc.vector.tensor_tensor(
                    out=out_sb[:, c * 256 : (c + 1) * 256],
                    in0=nd_ps[c][:, 0:256],
                    in1=recip[:, c * 256 : (c + 1) * 256],
                    op=mybir.AluOpType.mult,
                )
                nc.sync.dma_start(
                    out=out_ext[:, e * FD + c * 256 : e * FD + (c + 1) * 256],
                    in_=out_sb[:, c * 256 : (c + 1) * 256],
                )

        groups = [(e, dC) for e in range(EP) for dC in range(K)]
        pending = None  # (e, dC, lg) whose exp/prod/MMs are not yet emitted
        for e, dC in groups:
            lg = phase_logit(
                e, dC, dR_splits=[(0, 4), (4, K)] if (e, dC) == (0, 0) else None
            )
            if (e, dC) == (0, 0):
                # B planes are first needed by group (0,1); building them
                # here overlaps the chain with group (0,0)'s exp.
                strips(planesB, [(0, 2), (98, 102)])
                chain(planesB, 0, 14)
                q_wave(1)
            elif (e, dC) == (0, 1):
                # A-plane remainders (needed from epoch 1 on)
                chain(planesA, 11, SLAB, 1, 4)
                chain(planesA, 14, SLAB, 4, K)
                q_wave(2)
            elif (e, dC) == (0, 2):
                chain(planesB, 14, SLAB)
            if pending is not None:
                pe, pc, plg = pending
                phase_rest(pe, pc, plg, split=(pe == EP - 1 and pc >= K - 4))
                # normalize(e-1) is deferred until after group (e,0)'s rest:
                # its ln/exp would otherwise sit in the ACT queue AHEAD of
                # exp(e,0) and stall the next epoch's first e*v (~0.8us).
                if pc == 0 and pe > 0:
                    normalize(pe - 1)
            pending = (e, dC, lg)
        pe, pc, plg = pending
        phase_rest(pe, pc, plg, split=True)
        normalize(pe, per_chunk=True)

    _split_sync_waits(nc)
    return nc


def _host_prep(x, v, w_q, w_k, rel_h, rel_w):
    """Build the 8 per-core input maps (numpy only)."""
    x = np.asarray(x, np.float32)
    v = np.asarray(v, np.float32)
    w_q = np.asarray(w_q, np.float32)
    w_k = np.asarray(w_k, np.float32)
    rel_h = np.asarray(rel_h, np.float32).reshape(64, K)   # [c, i]
    rel_w = np.asarray(rel_w, np.float32).reshape(64, K)   # [c, j]

    ident = np.eye(128, dtype=np.float32).astype(ml_dtypes.bfloat16)

    in_maps = []
    for ci in range(NCORES):
        b, rest = divmod(ci, 4)
        half, t = divmod(rest, 2)
        ch0 = 64 * half
        if half == 0:
            xf = x[b]                          # [128, R=h, C=w]
            vf = v[b, ch0 : ch0 + 64]
            relv = rel_h                       # [c, m] with m = dR
        else:
            xf = np.ascontiguousarray(x[b].transpose(0, 2, 1))   # R=w, C=h
            vf = np.ascontiguousarray(v[b, ch0 : ch0 + 64].transpose(0, 2, 1))
            relv = rel_w

        R0 = RT * t
        # x slab: stored rows R0-3 .. R0+50, zero beyond the image
        xs = np.zeros((128, XSLAB, W), np.float32)
        glo, ghi = max(0, R0 - PAD), min(96, R0 + RT + PAD)
        xs[:, glo - (R0 - PAD) : ghi - (R0 - PAD), :] = xf[:, glo:ghi, :]

        # v family buffer: partition p = c + 64*rh
        vs = np.zeros((2, 64, SLAB, WPAD), np.float32)
        for rh in range(2):
            r0 = R0 + RH * rh
            lo, hi = max(0, r0 - PAD), min(96, r0 + RH + PAD)
            vs[rh, :, lo - (r0 - PAD) : hi - (r0 - PAD), PAD : PAD + W] = (
                vf[:, lo:hi, :]
            )
        vbufA = vs.reshape(128, SLAB, WPAD)
        vbufB = np.zeros_like(vbufA)
        vbufB[:, :, : WPAD - 1] = vbufA[:, :, 1:]

        relc = np.zeros((128, 8), np.float32)
        rv = np.concatenate([relv, relv], axis=0)          # [128, 7], p=c+64rh
        relc[:, 0] = rv[:, 0]
        relc[:, 1:K] = rv[:, 1:K] - rv[:, 0:1]

        # blocks: [weights | h0 slab rows 0-14 | h1 rows 24-38 |
        #          h0 rows 15-29 | h1 rows 39-53]
        xs_pack = np.empty((128, 128 + 4 * 15 * W), np.float16)
        xs_pack[:, 0:64] = w_k[ch0 : ch0 + 64].T.astype(np.float16)
        xs_pack[:, 64:128] = w_q[ch0 : ch0 + 64].T.astype(np.float16)
        xs16 = xs.astype(np.float16)
        for b, r0 in enumerate((0, 24, 15, 39)):
            xs_pack[:, 128 + b * 15 * W : 128 + (b + 1) * 15 * W] = xs16[
                :, r0 : r0 + 15, :
            ].reshape(128, 15 * W)
        in_maps.append(
            {
                "xs": np.ascontiguousarray(xs_pack),
                "vbufA": np.ascontiguousarray(vbufA.astype(ml_dtypes.bfloat16)),
                "vbufB": np.ascontiguousarray(vbufB.astype(ml_dtypes.bfloat16)),
                "relcols": relc,
                "ident": ident,
            }
        )
    return in_maps


def kernel(x, v, w_q, w_k, rel_h, rel_w, trace=False, tmpdir=None):
    from concourse.bass_utils import run_bass_kernel_spmd

    if "nc" not in _cache:
        _cache["nc"] = _build()
    nc = _cache["nc"]
    in_maps = _host_prep(x, v, w_q, w_k, rel_h, rel_w)
    res = run_bass_kernel_spmd(
        nc, in_maps, list(range(NCORES)), trace=trace, tmpdir=tmpdir
    )
    out = np.zeros((B, COUT, H, W), np.float32)
    for ci in range(NCORES):
        b, rest = divmod(ci, 4)
        half, t = divmod(rest, 2)
        ch0, R0 = 64 * half, RT * t
        a = res.results[ci]["out"].reshape(2, 64, EP, EP_ROWS, W)
        lines = a.transpose(1, 0, 2, 3, 4).reshape(64, RT, W)  # [c, line, C]
        if half == 0:
            out[b, ch0 : ch0 + 64, R0 : R0 + RT, :] = lines
        else:
            out[b, ch0 : ch0 + 64, :, R0 : R0 + RT] = lines.transpose(0, 2, 1)
    kernel.last_exec_time_ns = res.exec_time_ns
    kernel.last_results = res
    return out

